# revision 2
# baseline (speedup 1.0000x reference)
"""Builder for the AttnBlock Trainium2 kernel.

Layout strategy (per core: NB batches of NT tokens, C=512 channels):
  - LN1 computed token-major (bn_stats over free axis), h cast to bf16
  - h transposed to feature-major hT (DMA transpose by default; matmul
    contracts over the partition axis so both operands need C on partitions)
  - QKV projection split: q computed feature-major (qT = w_q^T @ hT),
    k/v computed token-major (kv = hT^T @ w_kv)
  - q softmax over d: exp on ACT during psum->sbuf copy; per-(token,head)
    sums via a packed ones-matmul; normalization applied by replicating
    1/S_q across partitions with a K=2 matmul and one DVE multiply
  - k softmax over n: exp only; the denominator S_k[d] = sum_n e_k[n,d]
    falls out of the context matmul via an appended ones-column on v
  - context[h] = e_k[h]^T @ [v[h] | 1] accumulated per 512-token chunk in
    PSUM (two heads packed in array column halves), folded into an SBUF
    accumulator; rows scaled by 1/(S_k * NT * 8) at bf16 cast
  - attn^T = context^T @ qnorm per head, two heads packed in diagonal
    array quadrants (partitions 0-63 / 64-127)
  - y = attn @ w_out token-major (+ b_out if nonzero), LN2 straight from
    PSUM, (* ln2_scale if non-unit), + x, DMA out
"""

from contextlib import ExitStack

import ml_dtypes
import numpy as np

import concourse.bass as bass
import concourse.bacc as bacc
import concourse.mybir as mybir
import concourse.tile as tile

P = 128
HEADS = 8
DHEAD = 64
C = 512
DIM = 512
F_QKV = 3 * DIM
EPS = 1e-5

FP32 = mybir.dt.float32
BF16 = mybir.dt.bfloat16
AF = mybir.ActivationFunctionType
ALU = mybir.AluOpType


def build_nc(n_b=2, n_tok=4096, use_bout=False, use_s2=False,
             transpose_mode="pe", pack_quadrants=True, rsqrt_mode="sqrt",
             vext_engine="act", attn_engine="act", mm_bufs=4, repeat=1, stage="full", ctx_bufs=2, sq_bufs=1, rep_bufs=1, fp8=True):
    """Build + compile the Bacc graph for one core handling [n_b, n_tok, C]."""
    nc = bacc.Bacc(
        "TRN2", target_bir_lowering=False, debug=False, enable_asserts=False
    )
    x_d = nc.dram_tensor("x", [n_b, n_tok, C], FP32, kind="ExternalInput").ap()
    wqkv_d = nc.dram_tensor("w_qkv", [C, F_QKV], FP32, kind="ExternalInput").ap()
    wout_d = nc.dram_tensor("w_out", [DIM, C], FP32, kind="ExternalInput").ap()
    bout_d = nc.dram_tensor("b_out", [C], FP32, kind="ExternalInput").ap()
    s2_d = nc.dram_tensor("ln2_scale", [C], FP32, kind="ExternalInput").ap()
    out_d = nc.dram_tensor("out", [n_b, n_tok, C], FP32, kind="ExternalOutput").ap()

    with tile.TileContext(nc) as tc:
        _body(tc, x_d, wqkv_d, wout_d, bout_d, s2_d, out_d, n_b, n_tok,
              use_bout, use_s2, transpose_mode, pack_quadrants, rsqrt_mode,
              vext_engine, attn_engine, mm_bufs, repeat, stage, ctx_bufs,
              sq_bufs, rep_bufs, fp8)
    nc.compile()
    return nc


def _body(tc, x_d, wqkv_d, wout_d, bout_d, s2_d, out_d, n_b, n_tok,
          use_bout, use_s2, transpose_mode, pack_quadrants, rsqrt_mode,
          vext_engine, attn_engine, mm_bufs, repeat=1, stage="full",
          ctx_bufs=2, sq_bufs=1, rep_bufs=1, fp8=False):

    def rsqrt(nc, out, var_ap, eps_t, scale=1.0, power=-0.5):
        # 1/sqrt(scale*var+eps) (power=-0.5) or 1/(scale*var) (power=-1)
        if rsqrt_mode == "lnexp":
            nc.scalar.activation(out, var_ap, AF.Ln, bias=eps_t, scale=scale)
            nc.scalar.activation(out, out, AF.Exp, scale=power)
        else:
            if power == -1.0:
                nc.scalar.mul(out, var_ap, scale)
                nc.vector.reciprocal(out, out)
            else:
                nc.scalar.activation(out, var_ap, AF.Sqrt, bias=eps_t,
                                     scale=scale)
                nc.vector.reciprocal(out, out)
    nc = tc.nc
    NCH = n_tok // 512          # 512-token chunks per batch
    CTX_SCALE = float(n_tok) * 8.0  # v/n and q/sqrt(dhead) folded together
    FP8 = mybir.dt.float8e4
    MMDT = FP8 if fp8 else BF16
    DR = mybir.MatmulPerfMode.DoubleRow if fp8 else None
    W_SC = 32.0 if fp8 else 1.0        # weight pre-scale into fp8 range
    CTX_UP = 1.0
    QN_UP = 1.0
    Y_DESC = 1.0 / (CTX_UP * QN_UP * W_SC)  # undo boosts after y matmul

    with ExitStack() as ctx:
        consts = ctx.enter_context(tc.tile_pool(name="consts", bufs=1))
        work = ctx.enter_context(tc.tile_pool(name="work", bufs=3))
        big = ctx.enter_context(tc.tile_pool(name="big", bufs=2))
        psum = ctx.enter_context(tc.tile_pool(name="psum", bufs=1, space="PSUM"))

        # ---- constants / weights ----
        if transpose_mode == "pe":
            # inline identity via the sync queue: keeps the first PE
            # transposes off the gpsimd queue that carries 4MB of weights
            id_np = np.eye(P, dtype=ml_dtypes.bfloat16)
            ident = consts.tile([P, P], BF16)
            nc.sync.dma_start(ident[:], nc.inline_tensor(id_np, "ident").ap())

        # w_qkv fp32 [c, f] -> bf16 SBUF, c on partitions in 4 chunks
        wq_f = consts.tile([P, 4, DIM], FP32)
        wkv_f = consts.tile([P, 4, 2 * DIM], FP32)
        wo_f = consts.tile([P, 4, C], FP32)
        # weight loads ride the gpsimd (SWDGE) queue so the first x tiles
        # don't wait behind 4MB of weights on the sync queue
        wq_r = wqkv_d.rearrange("(k p) f -> p k f", p=P)
        nc.gpsimd.dma_start(wq_f[:], wq_r[:, :, 0:DIM])
        nc.gpsimd.dma_start(wkv_f[:], wq_r[:, :, DIM:3 * DIM])
        nc.gpsimd.dma_start(wo_f[:], wout_d.rearrange("(k p) f -> p k f", p=P))
        wq = consts.tile([P, 4, DIM], MMDT)
        wkv = consts.tile([P, 4, 2 * DIM], MMDT)
        wo = consts.tile([P, 4, C], BF16)
        nc.vector.tensor_copy(wo[:], wo_f[:])
        if fp8:
            nc.vector.tensor_scalar_mul(wq[:], wq_f[:], W_SC)
            nc.vector.tensor_scalar_mul(wkv[:], wkv_f[:], W_SC)
        else:
            nc.vector.tensor_copy(wq[:], wq_f[:])
            nc.vector.tensor_copy(wkv[:], wkv_f[:])

        if use_bout:
            bout_bc = consts.tile([P, C], FP32)
            nc.sync.dma_start(bout_bc[:], bout_d[None, :].partition_broadcast(P))
        if use_s2:
            s2_bc = consts.tile([P, C], FP32)
            nc.sync.dma_start(s2_bc[:], s2_d[None, :].partition_broadcast(P))
        eps_t = consts.tile([P, 1], FP32)
        nc.vector.memset(eps_t[:], EPS)

        # ones for head-pair column sums: lhsT [128, 2]
        hp_np = np.zeros((P, 2), ml_dtypes.bfloat16)
        hp_np[0:64, 0] = 1.0 / QN_UP
        hp_np[64:128, 1] = 1.0 / QN_UP
        hp_ones = consts.tile([P, 2], BF16)
        nc.sync.dma_start(hp_ones[:], nc.inline_tensor(hp_np, "hp_ones").ap())
        # ones for replicating [2, t] -> [128, t]: lhsT [2, 128]
        cb_np = np.zeros((2, P), ml_dtypes.bfloat16)
        cb_np[0, 0:64] = 1
        cb_np[1, 64:128] = 1
        cb_ones = consts.tile([2, P], BF16)
        nc.sync.dma_start(cb_ones[:], nc.inline_tensor(cb_np, "cb_ones").ap())

        rep_cm = tc.For_i(0, repeat, 1) if repeat > 1 else None
        if rep_cm is not None:
            rep_cm.__enter__()
        # per-batch persistent tiles, both batches in flight (chunk-interleaved)
        expq_b = []
        ctx_acc_b = []
        ctx_bf_b = []
        for b in range(n_b):
            expq = big.tile([P, 4, NCH, 512], BF16, tag="expq")
            ctx_acc = big.tile([P, 4, DHEAD + 1], FP32, tag="ctx_acc")
            nc.vector.memset(ctx_acc[:], 0.0)
            expq_b.append(expq)
            ctx_acc_b.append(ctx_acc)

        # ---------------- pass 1 (batches interleaved per chunk) -------
        for tcn_b in range(NCH * n_b):
            tcn, b = divmod(tcn_b, n_b)
            expq = expq_b[b]
            ctx_acc = ctx_acc_b[b]
            if True:
                hT = big.tile([P, 4, 512], MMDT, tag="hT", bufs=2)
                ek_t = []
                vext_t = []
                for ti in range(4):
                    t0 = tcn * 512 + ti * 128
                    xt = work.tile([P, C], FP32, tag="x_in", bufs=5)
                    nc.sync.dma_start(xt[:], x_d[b, t0:t0 + 128, :])
                    # residual base: out = x now, z accumulated in pass 2
                    nc.sync.dma_start(out_d[b, t0:t0 + 128, :], xt[:])
                    stats = work.tile([P, 6], FP32, tag="bn_st", bufs=4)
                    nc.vector.bn_stats(stats[:], xt[:])
                    mv = work.tile([P, 2], FP32, tag="bn_mv", bufs=4)
                    nc.vector.bn_aggr(mv[:], stats[:])
                    rstd = work.tile([P, 1], FP32, tag="rstd", bufs=4)
                    rsqrt(nc, rstd[:], mv[:, 1:2], eps_t[:])
                    h_tm = work.tile([P, C], BF16, tag="h_tm", bufs=5)
                    nc.vector.tensor_scalar(
                        out=h_tm[:], in0=xt[:], scalar1=mv[:, 0:1],
                        scalar2=rstd[:], op0=ALU.subtract, op1=ALU.mult)
                    # transpose h tile into hT[:, :, ti*128:...]
                    if transpose_mode == "dma":
                        for ck in range(4):
                            nc.sync.dma_start(hT[:, ck, ti * 128:(ti + 1) * 128],
                                              h_tm[:, ck * P:(ck + 1) * P],
                                              transpose=True)
                    else:
                        # 4 transposes into one psum tile, drained by one copy
                        ps_tp = psum.tile([P, 4, P], BF16, tag="mm", bufs=mm_bufs)
                        for ck in range(4):
                            nc.tensor.transpose(ps_tp[:, ck, :],
                                                h_tm[:, ck * P:(ck + 1) * P],
                                                ident[:])
                        nc.vector.tensor_copy(
                            hT[:, :, ti * 128:(ti + 1) * 128], ps_tp[:])

                if stage == "ln1":
                    continue
                # q part: feature-major, 4 m-tiles of 128 dims (= head pairs)
                for m in range(4):
                    ps_q = psum.tile([P, 512], FP32, tag="mm", bufs=mm_bufs)
                    if fp8:
                        for k2 in (0, 2):
                            nc.tensor.matmul(
                                ps_q[:], wq[:, k2:k2 + 2, m * 128:(m + 1) * 128],
                                hT[:, k2:k2 + 2, :], start=(k2 == 0),
                                stop=(k2 == 2), perf_mode=DR)
                    else:
                        for k in range(4):
                            nc.tensor.matmul(
                                ps_q[:], wq[:, k, m * 128:(m + 1) * 128],
                                hT[:, k, :], start=(k == 0), stop=(k == 3))
                    eq = expq[:, m, tcn, :]
                    nc.scalar.activation(eq, ps_q[:], AF.Exp, scale=1.0 / W_SC)
                    # per-(token, head) sums over d (2 heads at once)
                    ps_sq = psum.tile([2, 512], FP32, tag="sq", bufs=sq_bufs)
                    nc.tensor.matmul(ps_sq[:], hp_ones[:], eq, start=True,
                                     stop=True)
                    rq_bf = work.tile([2, 512], BF16, tag="rq_bf", bufs=4)
                    with nc.allow_low_precision(reason="1/S_q in bf16 is fine"):
                        nc.vector.reciprocal(rq_bf[:], ps_sq[:])
                    ps_rep = psum.tile([P, 512], FP32, tag="rep", bufs=rep_bufs)
                    nc.tensor.matmul(ps_rep[:], cb_ones[:], rq_bf[:],
                                     start=True, stop=True)
                    nc.vector.tensor_tensor(eq, eq, ps_rep[:], ALU.mult)

                # k/v part: token-major [128t, 512f]
                for ti in range(4):
                    ek = work.tile([P, 512], BF16, tag="ek", bufs=8)
                    ps_k = psum.tile([P, 512], FP32, tag="mm", bufs=mm_bufs)
                    if fp8:
                        for k2 in (0, 2):
                            nc.tensor.matmul(
                                ps_k[:], hT[:, k2:k2 + 2, ti * 128:(ti + 1) * 128],
                                wkv[:, k2:k2 + 2, 0:512], start=(k2 == 0),
                                stop=(k2 == 2), perf_mode=DR)
                    else:
                        for k in range(4):
                            nc.tensor.matmul(
                                ps_k[:], hT[:, k, ti * 128:(ti + 1) * 128],
                                wkv[:, k, 0:512], start=(k == 0), stop=(k == 3))
                    nc.scalar.activation(ek[:], ps_k[:], AF.Exp,
                                         scale=1.0 / W_SC)
                    ek_t.append(ek)

                    vext = work.tile([P, HEADS, DHEAD + 1], BF16, tag="vext",
                                     bufs=9)
                    ps_v = psum.tile([P, 512], FP32, tag="mm", bufs=mm_bufs)
                    if fp8:
                        for k2 in (0, 2):
                            nc.tensor.matmul(
                                ps_v[:], hT[:, k2:k2 + 2, ti * 128:(ti + 1) * 128],
                                wkv[:, k2:k2 + 2, 512:1024], start=(k2 == 0),
                                stop=(k2 == 2), perf_mode=DR)
                    else:
                        for k in range(4):
                            nc.tensor.matmul(
                                ps_v[:], hT[:, k, ti * 128:(ti + 1) * 128],
                                wkv[:, k, 512:1024], start=(k == 0), stop=(k == 3))
                    if vext_engine == "act":
                        nc.scalar.mul(
                            vext[:, :, 0:DHEAD],
                            ps_v.rearrange("p (h e) -> p h e", h=HEADS),
                            1.0 / W_SC)
                    else:
                        nc.vector.tensor_scalar_mul(
                            vext[:, :, 0:DHEAD],
                            ps_v.rearrange("p (h e) -> p h e", h=HEADS),
                            1.0 / W_SC)
                    nc.vector.memset(vext[:, :, DHEAD:DHEAD + 1], 1.0)
                    vext_t.append(vext)

                if stage == "qkv":
                    continue
                # context accumulation (2 heads packed in array column halves;
                # psum tile is full-bank [128, 512] so partition-sliced
                # accumulation groups stay 2048B-row aligned)
                for hp in range(4):
                    ps_cx = psum.tile([P, 512], FP32, tag="ctx", bufs=ctx_bufs)
                    for ti in range(4):
                        ek = ek_t[ti]
                        he, ho = 2 * hp, 2 * hp + 1
                        nc.tensor.matmul(
                            ps_cx[0:64, 0:DHEAD + 1], ek[:, he * 64:he * 64 + 64],
                            vext_t[ti][:, he, :],
                            start=(ti == 0), stop=(ti == 3),
                            tile_position=(0, 0) if pack_quadrants else None,
                            skip_group_check=True)
                        nc.tensor.matmul(
                            ps_cx[64:128, 0:DHEAD + 1], ek[:, ho * 64:ho * 64 + 64],
                            vext_t[ti][:, ho, :],
                            start=(ti == 0), stop=(ti == 3),
                            tile_position=(0, 64) if pack_quadrants else None,
                            skip_group_check=True)
                    nc.vector.tensor_tensor(ctx_acc[:, hp, :], ctx_acc[:, hp, :],
                                            ps_cx[:, 0:DHEAD + 1], ALU.add)

        if stage in ("ln1", "qkv", "p1"):
            if rep_cm is not None:
                rep_cm.__exit__(None, None, None)
            return
        # ---------------- context finalize ----------------
        for b in range(n_b):
            ctx_acc = ctx_acc_b[b]
            ctx_bf = big.tile([P, 4, DHEAD], BF16, tag="ctx_bf")
            ctx_bf_b.append(ctx_bf)
            for hp in range(4):
                s_col = work.tile([P, 1], FP32, tag="sk", bufs=2)
                rsqrt(nc, s_col[:], ctx_acc[:, hp, DHEAD:DHEAD + 1], eps_t[:],
                      scale=CTX_SCALE / CTX_UP, power=-1.0)
                nc.vector.tensor_scalar_mul(
                    ctx_bf[:, hp, :], ctx_acc[:, hp, 0:DHEAD], s_col[:])

        # ---------------- pass 2 (batches interleaved per chunk) -------
        for tcn_b in range(NCH * n_b):
            tcn, b = divmod(tcn_b, n_b)
            expq = expq_b[b]
            ctx_bf = ctx_bf_b[b]
            if True:
                at8 = work.tile([P, 4, 512], BF16, tag="attn", bufs=2)
                for hp in range(4):
                    ps_at = psum.tile([P, 512], FP32, tag="mm", bufs=mm_bufs)
                    nc.tensor.matmul(
                        ps_at[0:64, :], ctx_bf[0:64, hp, :],
                        expq[0:64, hp, tcn, :], start=True, stop=True,
                        tile_position=(0, 0) if pack_quadrants else None,
                        skip_group_check=True)
                    nc.tensor.matmul(
                        ps_at[64:128, :], ctx_bf[64:128, hp, :],
                        expq[64:128, hp, tcn, :], start=True, stop=True,
                        tile_position=(64, 64), skip_group_check=True)
                    if attn_engine == "act":
                        nc.scalar.copy(at8[:, hp, :], ps_at[:])
                    else:
                        nc.vector.tensor_copy(at8[:, hp, :], ps_at[:])

                for ts in range(4):
                    t0 = tcn * 512 + ts * 128
                    ps_y = psum.tile([P, 512], FP32, tag="mm", bufs=mm_bufs)
                    for hp in range(4):
                        nc.tensor.matmul(
                            ps_y[:], at8[:, hp, ts * 128:(ts + 1) * 128],
                            wo[:, hp, :], start=(hp == 0), stop=(hp == 3))
                    y_src = work.tile([P, C], FP32, tag="y_sb", bufs=4)
                    if use_bout:
                        nc.vector.tensor_tensor(y_src[:], ps_y[:], bout_bc[:],
                                                ALU.add)
                    elif ts % 2 == 0:
                        nc.scalar.copy(y_src[:], ps_y[:])
                    else:
                        nc.vector.tensor_copy(y_src[:], ps_y[:])
                    stats2 = work.tile([P, 6], FP32, tag="bn_st2", bufs=4)
                    nc.vector.bn_stats(stats2[:], y_src[:])
                    mv2 = work.tile([P, 2], FP32, tag="bn_mv2", bufs=4)
                    nc.vector.bn_aggr(mv2[:], stats2[:])
                    r2 = work.tile([P, 1], FP32, tag="r2", bufs=4)
                    rsqrt(nc, r2[:], mv2[:, 1:2], eps_t[:])
                    nmr2 = work.tile([P, 1], FP32, tag="nmr2", bufs=4)
                    nc.vector.tensor_scalar(
                        out=nmr2[:], in0=mv2[:, 0:1], scalar1=r2[:],
                        scalar2=-1.0, op0=ALU.mult, op1=ALU.mult)
                    z = work.tile([P, C], FP32, tag="z", bufs=4)
                    nc.scalar.activation(z[:], y_src[:], AF.Identity,
                                         bias=nmr2[:], scale=r2[:])
                    if use_s2:
                        nc.vector.tensor_tensor(z[:], z[:], s2_bc[:], ALU.mult)
                    nc.gpsimd.dma_start(out_d[b, t0:t0 + 128, :], z[:],
                                        accum_op=ALU.add)

        if rep_cm is not None:
            rep_cm.__exit__(None, None, None)



# ---------------------------------------------------------------------------
# kernel(): full-input entry point. Shards batch over 8 NeuronCores,
# folds ln1_scale into w_qkv on the host, runs the SPMD NEFF, regathers.
# ---------------------------------------------------------------------------

N_CORES = 8
B_FULL = 16
H_IMG = 64
W_IMG = 64
NB_PER_CORE = B_FULL // N_CORES
N_TOK = H_IMG * W_IMG

_nc_cache = {}


def _get_nc(use_bout, use_s2):
    key = (use_bout, use_s2)
    if key not in _nc_cache:
        _nc_cache[key] = build_nc(n_b=NB_PER_CORE, n_tok=N_TOK,
                                  use_bout=use_bout, use_s2=use_s2)
    return _nc_cache[key]


def kernel(x, ln1_scale, w_qkv, w_out, b_out, ln2_scale):
    from concourse.bass_utils import run_bass_kernel_spmd

    x = np.ascontiguousarray(np.asarray(x, dtype=np.float32))
    ln1_scale = np.asarray(ln1_scale, dtype=np.float32)
    w_qkv = np.asarray(w_qkv, dtype=np.float32)
    w_out = np.ascontiguousarray(np.asarray(w_out, dtype=np.float32))
    b_out = np.ascontiguousarray(np.asarray(b_out, dtype=np.float32))
    ln2_scale = np.ascontiguousarray(np.asarray(ln2_scale, dtype=np.float32))

    # fold ln1_scale into the qkv weight (h*s1 @ w == h @ (s1[:,None]*w))
    w_eff = np.ascontiguousarray(ln1_scale[:, None] * w_qkv)

    use_bout = bool(np.any(b_out))
    use_s2 = not bool(np.all(ln2_scale == 1.0))
    nc = _get_nc(use_bout, use_s2)

    xr = x.reshape(B_FULL, N_TOK, C)
    in_maps = []
    for i in range(N_CORES):
        in_maps.append({
            "x": np.ascontiguousarray(xr[i * NB_PER_CORE:(i + 1) * NB_PER_CORE]),
            "w_qkv": w_eff,
            "w_out": w_out,
            "b_out": b_out,
            "ln2_scale": ln2_scale,
        })

    res = run_bass_kernel_spmd(nc, in_maps, core_ids=list(range(N_CORES)))
    out = np.concatenate([r["out"] for r in res.results], axis=0)
    return out.reshape(B_FULL, H_IMG, W_IMG, C).astype(np.float32)

